# revision 6
# baseline (speedup 1.0000x reference)
"""CGCNN Interactions (NNConv message passing) on 8 TRN2 NeuronCores, v2.

Strategy (edge-parallel, sharded by destination-node range, affine-z
decomposition):
  - core m owns nodes [m*1250, (m+1)*1250) and ALL edges whose dst is there.
  - z = relu(nn1(a1)) is affine in a1 = relu(short(edge_attr)) (3-dim);
    edges are classified by the sign pattern of a1 (7 classes).  Per class,
    channels that are (almost) always on are folded exactly into per-class
    affine weights C_k / D_{k,dim}; only genuinely mixed channels keep an
    explicit correction row (relu(+-pre)), pruned below TAU.
  - per tile of 512 edges the message is
        msg^T = sum_chunk C_cls^T x^T  +  sum_pairs Wpair^T (rep ⊙ xts)
    with only ~1.3 rep-pairs per tile on average (40% of edges need just
    the const matmul).
  - edges are sorted by dst inside each class run, so the one-hot
    mean-scatter matmul per 128-edge chunk only spans a narrow dst window.
  - node features exchanged via AllGather (bf16) between iterations.

kernel(**inputs) takes FULL inputs, shards on host, runs one NEFF on cores
0..7 via run_bass_kernel_spmd, and reassembles the full [10000, 64] output.
"""

import math
from contextlib import ExitStack

import numpy as np
import ml_dtypes

import concourse.bass as bass
import concourse.bacc as bacc
import concourse.tile as tile
import concourse.mybir as mybir
from concourse.bass import IndirectOffsetOnAxis
from concourse.bass_utils import run_bass_kernel_spmd
from concourse.masks import make_identity

BF16 = mybir.dt.bfloat16
F32 = mybir.dt.float32
FP8 = mybir.dt.float8e4
I32 = mybir.dt.int32
I16 = mybir.dt.int16
NPBF16 = ml_dtypes.bfloat16

# problem constants
N = 10000
E = 50000
HC = 64
NF = 64
NCORES = 8
NPC = N // NCORES          # 1250 nodes owned per core
NPAD = 1280                # padded to 10 x 128 rows
BLK = 512                  # agg psum bank width (3 banks cover 1250 nodes)
NBLK = 3
N_CONV = int(__import__('os').environ.get('NCONV', '2'))
TAU = 0.012                # correction-row pruning threshold

ALL_CORES = list(range(NCORES))
CLASS_PATS = [(0,), (4,), (2,), (1,), (5,), (3,), (6, 7)]
NCLS = len(CLASS_PATS)


# ---------------------------------------------------------------- host prep

def _prep(inputs):
    src = np.asarray(inputs["edge_index"])[0].astype(np.int64)
    dst = np.asarray(inputs["edge_index"])[1].astype(np.int64)
    ea5 = np.asarray(inputs["edge_attr"], dtype=np.float32)
    sw = np.asarray(inputs["short_w"], np.float32)
    sb = np.asarray(inputs["short_b"], np.float32)
    nn1w = np.asarray(inputs["nn1_w"], np.float32)
    nn1b = np.asarray(inputs["nn1_b"], np.float32)
    W2 = np.asarray(inputs["nn2_w"], np.float32).reshape(HC, HC, NF)
    b2 = np.asarray(inputs["nn2_b"], np.float32).reshape(HC, NF)

    a1 = np.maximum(ea5 @ sw + sb, 0.0)          # [E,3]
    pre = a1 @ nn1w + nn1b                       # [E,64]
    pat = a1 > 0
    patid = pat[:, 0] * 4 + pat[:, 1] * 2 + pat[:, 2]
    clsid = np.zeros(E, np.int64)
    for k, ps in enumerate(CLASS_PATS):
        clsid[np.isin(patid, ps)] = k

    # per-class fold sets, affine weights, correction rows
    cls_fold = []
    cls_adims = []
    cls_corr = []          # list of dict ch -> per-edge values (class edges)
    constw = np.zeros((NCLS, HC, NF), np.float32)
    dw = {}                # (k, dim) -> [HC, NF]
    for k in range(NCLS):
        m = clsid == k
        prem = pre[m]
        fold = prem.mean(0) > 0          # mostly-on channels -> fold
        fold = (prem > 0).mean(0) >= 0.5
        corr = {}
        for c in range(HC):
            if fold[c]:
                qc = np.maximum(-prem[:, c], 0.0)
                if qc.max() > TAU:
                    corr[c] = qc
            else:
                zc = np.maximum(prem[:, c], 0.0)
                if zc.max() > TAU:
                    corr[c] = zc
        adims = sorted(set(np.where(np.any(pat[m], axis=0))[0].tolist())) \
            if m.sum() else []
        cls_fold.append(fold)
        cls_adims.append(adims)
        cls_corr.append(corr)
        constw[k] = np.einsum('c,cio->io', nn1b * fold, W2) + b2
        for dim in adims:
            dw[(k, dim)] = np.einsum('c,cio->io', nn1w[dim] * fold, W2)

    # per-edge correction value lookup (E x HC sparse-ish, dense is fine)
    corrval = np.zeros((E, HC), np.float32)
    for k in range(NCLS):
        m = clsid == k
        for c, v in cls_corr[k].items():
            corrval[m, c] = v

    # ---- per-core sort by (class, dstloc) and shared run lengths
    core = dst // NPC
    dstloc = dst - core * NPC
    cnt = np.bincount(dst, minlength=N).astype(np.float32)
    invc_all = (1.0 / np.maximum(cnt, 1.0))[dst].astype(np.float32)
    srcrow_all = ((src // NPC) * NPAD + (src % NPC)).astype(np.int32)

    run_len = np.zeros(NCLS, np.int64)
    for k in range(NCLS):
        cnts = np.array([((core == m) & (clsid == k)).sum()
                         for m in range(NCORES)])
        run_len[k] = 128 * int(np.ceil(cnts.max() / 128))
    epad = int(run_len.sum())
    epad = int(np.ceil(epad / 512) * 512)
    nchunk = epad // 128
    ntile = epad // 512
    bound = np.concatenate([[0], np.cumsum(run_len)]).astype(np.int64)

    chunk_cls = np.zeros(nchunk, np.int64)     # pad region -> class 0
    for k in range(NCLS):
        chunk_cls[bound[k] // 128: bound[k + 1] // 128] = k

    # per-core edge placement
    srcrow_a = np.zeros((NCORES, epad), np.int32)
    dstloc_a = np.full((NCORES, epad), -1, np.int64)
    invc_a = np.zeros((NCORES, epad), np.float32)
    eidx_a = np.full((NCORES, epad), -1, np.int64)   # original edge index
    for m in range(NCORES):
        for k in range(NCLS):
            sel = np.nonzero((core == m) & (clsid == k))[0]
            sel = sel[np.argsort(dstloc[sel], kind='stable')]
            o = int(bound[k])
            n = len(sel)
            srcrow_a[m, o:o + n] = srcrow_all[sel]
            dstloc_a[m, o:o + n] = dstloc[sel]
            invc_a[m, o:o + n] = invc_all[sel]
            eidx_a[m, o:o + n] = sel

    # ---- scatter windows per chunk (core-invariant), sone + segments
    win_lo = np.zeros(nchunk, np.int64)
    win_hi = np.zeros(nchunk, np.int64)
    for c in range(nchunk):
        d = dstloc_a[:, c * 128:(c + 1) * 128]
        valid = d >= 0
        if valid.any():
            win_lo[c] = d[valid].min()
            win_hi[c] = d[valid].max() + 1
        else:
            win_lo[c], win_hi[c] = 0, 1
    sone_off = np.zeros(nchunk, np.int64)
    off = 0
    for c in range(nchunk):
        sone_off[c] = off
        off += int(win_hi[c] - win_lo[c])
    SW = int(off)
    sone = np.zeros((NCORES, 128, SW), NPBF16)
    for m in range(NCORES):
        for c in range(nchunk):
            d = dstloc_a[m, c * 128:(c + 1) * 128]
            v = invc_a[m, c * 128:(c + 1) * 128]
            p = np.nonzero(d >= 0)[0]
            sone[m, p, sone_off[c] + d[p] - win_lo[c]] = v[p]
    # scatter segments: (chunk, bank, out_lo, out_wid, sone_lo)
    segs = []
    for c in range(nchunk):
        lo, hi = int(win_lo[c]), int(win_hi[c])
        b0, b1 = lo // BLK, (hi - 1) // BLK
        for b in range(b0, b1 + 1):
            s = max(lo, b * BLK)
            e = min(hi, (b + 1) * BLK)
            segs.append((c, b, s - b * BLK, e - s, int(sone_off[c]) + s - lo))
    # last scatter segment per bank (for stop flag)
    last_seg_of_bank = {}
    for i, (c, b, *_rest) in enumerate(segs):
        last_seg_of_bank[b] = i

    # ---- per-tile plans: row lists -> pairs
    rows_of_tile = []
    for t in range(ntile):
        cls_here = sorted(set(chunk_cls[4 * t:4 * t + 4].tolist()))
        rows = []
        for k in cls_here:
            for dim in cls_adims[k]:
                rows.append(('a', k, dim))
        cu = sorted(set().union(*[set(cls_corr[k].keys()) for k in cls_here]))
        for ch in cu:
            rows.append(('c', ch))
        rows_of_tile.append(rows)
    pair_of_tile = []       # list of (slot_start, n_pairs)
    pair_rows = []          # flat list of (row1, row2|None) per slot
    for t in range(ntile):
        rows = rows_of_tile[t]
        s0 = len(pair_rows)
        for i in range(0, len(rows), 2):
            r2 = rows[i + 1] if i + 1 < len(rows) else None
            pair_rows.append((rows[i], r2))
        pair_of_tile.append((s0, len(pair_rows) - s0))
    NP = len(pair_rows)

    # ---- REP [NCORES, 128, NP*512], WPAIR [128, NP*64]
    def rowvals(m, t, row):
        e = eidx_a[m, t * 512:(t + 1) * 512]
        cc = chunk_cls[4 * t:4 * t + 4]
        out = np.zeros(512, np.float32)
        ok = e >= 0
        if row[0] == 'a':
            _, k, dim = row
            kmask = np.repeat(cc == k, 128) & ok
            out[kmask] = a1[e[kmask], dim]
        else:
            _, ch = row
            out[ok] = corrval[e[ok], ch]
        return out

    def roww(row):
        if row is None:
            return np.zeros((HC, NF), np.float32)
        if row[0] == 'a':
            _, k, dim = row
            return dw[(k, dim)]
        return W2[row[1]]

    rep = np.zeros((NCORES, 128, NP * 512), NPBF16)
    wpair = np.zeros((128, NP * NF), NPBF16)
    for j, (r1, r2) in enumerate(pair_rows):
        t = next(tt for tt in range(ntile)
                 if pair_of_tile[tt][0] <= j < sum(pair_of_tile[tt]))
        for m in range(NCORES):
            rep[m, 0:64, j * 512:(j + 1) * 512] = rowvals(m, t, r1)[None, :]
            if r2 is not None:
                rep[m, 64:128, j * 512:(j + 1) * 512] = \
                    rowvals(m, t, r2)[None, :]
        wpair[0:64, j * NF:(j + 1) * NF] = roww(r1)
        wpair[64:128, j * NF:(j + 1) * NF] = roww(r2)

    def dev128(a):  # [.., epad] -> [.., 128, nchunk] (p = e%128)
        return np.ascontiguousarray(
            a.reshape(a.shape[:-1] + (nchunk, 128)).swapaxes(-1, -2))

    h = np.asarray(inputs["h"], np.float32)
    hT_own = np.zeros((NCORES, HC, NPAD), np.float32)
    for m in range(NCORES):
        hT_own[m, :, :NPC] = h[m * NPC:(m + 1) * NPC].T

    w = {
        "lin0_w": np.asarray(inputs["lin0_w"], np.float32),
        "lin0_b": np.asarray(inputs["lin0_b"], np.float32)[:, None],
        "root_w": np.asarray(inputs["root_w"], np.float32),
        "conv_b": np.asarray(inputs["conv_b"], np.float32)[:, None],
        "constw": np.ascontiguousarray(
            constw.transpose(1, 0, 2).reshape(HC, NCLS * NF)).astype(NPBF16),
        "wpair": wpair,
    }
    meta = dict(epad=epad, nchunk=nchunk, ntile=ntile, NP=NP, SW=SW,
                chunk_cls=chunk_cls, pair_of_tile=pair_of_tile, segs=segs,
                last_seg_of_bank=last_seg_of_bank)
    per_core = dict(
        srcrow=dev128(srcrow_a),          # [8,128,nchunk] i32
        sone=sone,                        # [8,128,SW] bf16
        rep=rep,                          # [8,128,NP*512] bf16
        hT_own=hT_own,                    # [8,64,1280] f32
    )
    return meta, per_core, w


# ---------------------------------------------------------------- program

def _build(meta):
    epad = meta["epad"]
    nchunk = meta["nchunk"]
    ntile = meta["ntile"]
    NP = meta["NP"]
    SW = meta["SW"]
    chunk_cls = meta["chunk_cls"]
    pair_of_tile = meta["pair_of_tile"]
    segs = meta["segs"]
    last_seg_of_bank = meta["last_seg_of_bank"]

    nc = bacc.Bacc("TRN2", target_bir_lowering=False, debug=False,
                   enable_asserts=False, num_devices=NCORES)

    t_in = {}
    t_in["srcrow"] = nc.dram_tensor("srcrow", [128, nchunk], I32,
                                    kind="ExternalInput")
    t_in["sone"] = nc.dram_tensor("sone", [128, SW], BF16,
                                  kind="ExternalInput")
    t_in["rep"] = nc.dram_tensor("rep", [128, NP * 512], BF16,
                                 kind="ExternalInput")
    t_in["hT_own"] = nc.dram_tensor("hT_own", [HC, NPAD], F32,
                                    kind="ExternalInput")
    t_in["lin0_w"] = nc.dram_tensor("lin0_w", [HC, NF], F32,
                                    kind="ExternalInput")
    t_in["lin0_b"] = nc.dram_tensor("lin0_b", [NF, 1], F32,
                                    kind="ExternalInput")
    t_in["root_w"] = nc.dram_tensor("root_w", [NF, NF], F32,
                                    kind="ExternalInput")
    t_in["conv_b"] = nc.dram_tensor("conv_b", [NF, 1], F32,
                                    kind="ExternalInput")
    t_in["constw"] = nc.dram_tensor("constw", [HC, NCLS * NF], BF16,
                                    kind="ExternalInput")
    t_in["wpair"] = nc.dram_tensor("wpair", [128, NP * NF], BF16,
                                   kind="ExternalInput")

    out_own = nc.dram_tensor("out_own", [NPAD, NF], F32,
                             kind="ExternalOutput")
    own_rows = nc.dram_tensor("own_rows", [NPAD, NF], BF16)
    outbuf = nc.dram_tensor("outbuf", [NCORES * NPAD, NF], BF16,
                            addr_space="Shared")

    with tile.TileContext(nc) as tc, ExitStack() as ctx:
        cp = ctx.enter_context(tc.tile_pool(name="const", bufs=1))
        wp = ctx.enter_context(tc.tile_pool(name="work", bufs=3))
        pxt = ctx.enter_context(tc.tile_pool(name="pxt", bufs=2,
                                              space="PSUM"))
        pmsg = ctx.enter_context(tc.tile_pool(name="pmsg", bufs=2,
                                              space="PSUM"))
        pmr = ctx.enter_context(tc.tile_pool(name="pmr", bufs=2, space="PSUM"))
        pagg = ctx.enter_context(tc.tile_pool(name="pagg", bufs=1,
                                              space="PSUM"))

        def cload(name, shape, dtype):
            t = cp.tile(shape, dtype, tag=name)
            nc.sync.dma_start(t[:], t_in[name].ap())
            return t

        srcrow_s = cload("srcrow", [128, nchunk], I32)
        sone_s = cload("sone", [128, SW], BF16)
        rep_s = cload("rep", [128, NP * 512], BF16)
        hT_s = cload("hT_own", [HC, NPAD], F32)
        lin0w_s = cload("lin0_w", [HC, NF], F32)
        lin0b_s = cload("lin0_b", [NF, 1], F32)
        rootw_s = cload("root_w", [NF, NF], F32)
        convb_s = cload("conv_b", [NF, 1], F32)
        constw_s = cload("constw", [HC, NCLS * NF], BF16)
        wpair_s = cload("wpair", [128, NP * NF], BF16)

        ident_f = cp.tile([128, 128], F32, tag="identf")
        make_identity(nc, ident_f[:])
        ident_bf = cp.tile([128, 128], BF16, tag="identb")
        make_identity(nc, ident_bf[:])

        outT = [cp.tile([64, NPAD], F32, tag=f"outT{i}", name=f"outT{i}")
                for i in range(2)]

        col_groups = [(slice(0, 512), 512), (slice(512, 1024), 512),
                      (slice(1024, NPAD), NPAD - 1024)]

        def tail_broadcast(oT, last):
            # PE-transpose per 128-node block into one SBUF rows tile, then a
            # single DMA to DRAM (rows laid out [128, nb, 64] = node p+128*nb)
            NB = NPAD // 128
            rdt = F32 if last else BF16
            rows = wp.tile([128, NB * NF], rdt, tag="rows_f" if last
                           else "rows_8")
            for nb in range(NB):
                p_r = pmr.tile([128, 4 * NF], F32, tag="mr")
                nc.tensor.transpose(out=p_r[:, :NF],
                                    in_=oT[:, 128 * nb:128 * (nb + 1)],
                                    identity=ident_f[:64, :64])
                if nb % 2:
                    nc.vector.tensor_copy(rows[:, NF * nb:NF * (nb + 1)],
                                          p_r[:, :NF])
                else:
                    nc.scalar.activation(rows[:, NF * nb:NF * (nb + 1)],
                                         p_r[:, :NF],
                                         mybir.ActivationFunctionType.Copy)
            tgt = out_own if last else own_rows
            nc.sync.dma_start(
                tgt.ap().rearrange("(g p) i -> p g i", p=128), rows[:])
            if not last:
                nc.gpsimd.collective_compute(
                    "AllGather", mybir.AluOpType.bypass,
                    replica_groups=[ALL_CORES],
                    ins=[own_rows.ap()], outs=[outbuf.ap()])

        # ---- out0 = relu(lin0^T hT + b)
        for sl, n in col_groups:
            p_o = pmsg.tile([64, 512], F32, tag="msg")
            nc.tensor.matmul(p_o[:, :n], lhsT=lin0w_s[:], rhs=hT_s[:, sl],
                             start=True, stop=True, skip_group_check=True)
            nc.scalar.activation(outT[0][:, sl], p_o[:, :n],
                                 mybir.ActivationFunctionType.Relu,
                                 bias=lin0b_s[:])
        tail_broadcast(outT[0], last=False)

        # ---- conv iterations
        for it in range(N_CONV):
            cur = outT[it % 2]
            nxt = outT[(it + 1) % 2]

            # agg banks: root^T cur opens the accumulation (start=True)
            # blocks 0/1 share one PSUM bank at partition ranges 0-63/64-127
            p_aggA = pagg.tile([128, BLK], F32, tag="aggA",
                               name=f"aggA{it}")
            p_aggB = pagg.tile([64, BLK], F32, tag="aggB", name=f"aggB{it}")
            aggb = [p_aggA[0:64, :], p_aggA[64:128, :], p_aggB[:, :]]
            for b in range(NBLK):
                sl, n = col_groups[b]
                nc.tensor.matmul(aggb[b][:, :n], lhsT=rootw_s[:],
                                 rhs=cur[:, sl], start=True, stop=False,
                                 skip_group_check=True)

            for t in range(ntile):
                xg = wp.tile([128, 4 * NF], BF16, tag="xg")
                for c4 in range(4):
                    ch = 4 * t + c4
                    nc.gpsimd.indirect_dma_start(
                        out=xg[:, NF * c4:NF * (c4 + 1)], out_offset=None,
                        in_=outbuf.ap(),
                        in_offset=IndirectOffsetOnAxis(
                            ap=srcrow_s[:, ch:ch + 1], axis=0))
                p_xt = pxt.tile([64, 512], BF16, tag="xt")
                for c4 in range(4):
                    nc.tensor.transpose(
                        out=p_xt[:, 128 * c4:128 * (c4 + 1)],
                        in_=xg[:, NF * c4:NF * (c4 + 1)],
                        identity=ident_bf[:])
                s0, npair = pair_of_tile[t]
                cc = chunk_cls[4 * t:4 * t + 4]
                pure = cc[0] == cc[1] == cc[2] == cc[3]

                if npair == 0:
                    # const-only tile: msg rows directly per chunk
                    xts = wp.tile([64, 512], BF16, tag="xts0")
                    nc.vector.tensor_copy(xts[:], p_xt[:])
                    p_m = pmr.tile([128, 4 * NF], F32, tag="mr")
                    for c4 in range(4):
                        k = int(cc[c4])
                        nc.tensor.matmul(
                            p_m[:, NF * c4:NF * (c4 + 1)],
                            lhsT=xts[:, 128 * c4:128 * (c4 + 1)],
                            rhs=constw_s[:, NF * k:NF * (k + 1)],
                            start=(c4 == 0), stop=(c4 == 3),
                            skip_group_check=True)
                else:
                    xts = wp.tile([128, 512], BF16, tag="xts")
                    nc.scalar.activation(xts[:64, :], p_xt[:],
                                         mybir.ActivationFunctionType.Copy)
                    nc.sync.dma_start(xts[64:, :], xts[:64, :])
                    p_msg = pmsg.tile([64, 512], F32, tag="msg")
                    # const first (opens accumulation per slice)
                    if pure:
                        k = int(cc[0])
                        nc.tensor.matmul(
                            p_msg[:], lhsT=constw_s[:, NF * k:NF * (k + 1)],
                            rhs=xts[:64, :], start=True, stop=False,
                            skip_group_check=True)
                    else:
                        for c4 in range(4):
                            k = int(cc[c4])
                            nc.tensor.matmul(
                                p_msg[:, 128 * c4:128 * (c4 + 1)],
                                lhsT=constw_s[:, NF * k:NF * (k + 1)],
                                rhs=xts[:64, 128 * c4:128 * (c4 + 1)],
                                start=(c4 == 0), stop=False,
                                skip_group_check=True)
                    for j in range(s0, s0 + npair):
                        u = wp.tile([128, 512], BF16, tag="u")
                        nc.vector.tensor_tensor(
                            out=u[:], in0=rep_s[:, 512 * j:512 * (j + 1)],
                            in1=xts[:], op=mybir.AluOpType.mult)
                        nc.tensor.matmul(
                            p_msg[:], lhsT=wpair_s[:, NF * j:NF * (j + 1)],
                            rhs=u[:], start=False,
                            stop=(j == s0 + npair - 1),
                            skip_group_check=True)
                    msgs = wp.tile([64, 512], F32, tag="msgs")
                    nc.scalar.activation(msgs[:], p_msg[:],
                                         mybir.ActivationFunctionType.Copy)
                    p_m = pmr.tile([128, 4 * NF], F32, tag="mr")
                    for c4 in range(4):
                        nc.tensor.transpose(
                            out=p_m[:, NF * c4:NF * (c4 + 1)],
                            in_=msgs[:, 128 * c4:128 * (c4 + 1)],
                            identity=ident_f[:64, :64])

                msgr = wp.tile([128, 4 * NF], BF16, tag="msgr")
                nc.scalar.activation(msgr[:], p_m[:],
                                     mybir.ActivationFunctionType.Copy)

                # scatter segments of this tile's 4 chunks
                for si, (c, b, out_lo, wid, s_lo) in enumerate(segs):
                    if not (4 * t <= c < 4 * t + 4):
                        continue
                    c4 = c - 4 * t
                    nc.tensor.matmul(
                        aggb[b][:, out_lo:out_lo + wid],
                        lhsT=msgr[:, NF * c4:NF * (c4 + 1)],
                        rhs=sone_s[:, s_lo:s_lo + wid],
                        start=False, stop=(last_seg_of_bank[b] == si),
                        skip_group_check=True)

            # node update: out' = relu(agg_psum + b)  (root part already in)
            for b, (sl, n) in enumerate(col_groups):
                nc.scalar.activation(nxt[:, sl], aggb[b][:, :n],
                                     mybir.ActivationFunctionType.Relu,
                                     bias=convb_s[:])
            tail_broadcast(nxt, last=(it == N_CONV - 1))

    nc.compile()
    return nc


_CACHE = {}


def _meta_key(meta):
    return (meta["epad"], meta["NP"], meta["SW"],
            tuple(meta["chunk_cls"].tolist()),
            tuple(meta["pair_of_tile"]),
            tuple(meta["segs"]))


def _get_nc(meta):
    key = _meta_key(meta)
    if key not in _CACHE:
        _CACHE[key] = _build(meta)
    return _CACHE[key]


def _in_maps(meta, per_core, w):
    maps = []
    for m in range(NCORES):
        d = {
            "srcrow": per_core["srcrow"][m],
            "sone": per_core["sone"][m],
            "rep": per_core["rep"][m],
            "hT_own": per_core["hT_own"][m],
        }
        for k in ("lin0_w", "lin0_b", "root_w", "conv_b", "constw",
                  "wpair"):
            d[k] = w[k]
        maps.append(d)
    return maps


def _run(inputs, trace=False):
    meta, per_core, w = _prep(inputs)
    nc = _get_nc(meta)
    res = run_bass_kernel_spmd(nc, _in_maps(meta, per_core, w), ALL_CORES,
                               trace=trace)
    out = np.concatenate(
        [res.results[m]["out_own"][:NPC] for m in range(NCORES)], axis=0)
    return out.astype(np.float32), res


def kernel(**inputs):
    out, _ = _run(inputs, trace=False)
    return out
